# revision 14
# baseline (speedup 1.0000x reference)
"""Sliding-window attention (RoPE + QKV proj + windowed softmax attention + o_proj)
for Trainium2, SPMD over 8 NeuronCores.

Sharding: batch (2) x head-groups (4 groups of 4 heads) -> 8 cores.
Each core computes qkv for its 4 heads, windowed attention, and a partial
o_proj (its heads' columns of w_o); host sums the 4 partials per batch.

Precision scheme:
- QKV proj and o_proj run in fp8 e4m3 with DoubleRow perf mode (2x128
  contraction per instruction at 0.5 cyc/row) using a 3-product hi/lo
  residual decomposition: x*w ~= xh*wh + xl*wh + xh*wl, which costs
  0.75 cyc/row effective and has ~bf16-level error (the dropped xl*wl
  term is O(eps^2)).
- Weights are pre-scaled by 64 and x by 4 (host side) so the fp8 lo
  residuals stay above the e4m3 subnormal flush threshold; the 1/256 is
  folded into the rope tables / evacuation scales.
- The attention phase (scores, softmax, PV) runs in bf16.

Rope'd q/k and v stay resident in SBUF between the QKV and attention
phases (no DRAM spill).
"""
import sys

sys.path.insert(0, "/opt/trn_rl_repo")

import numpy as np

B = 2
S = 2048
HIDDEN = 2048
N_HEADS = 16
DH = 128
WINDOW = 512
HPC = 4  # heads per core
N_CORES = 8
SCALE = 1.0 / np.sqrt(DH)
NEG = -1.0e30
XS = 4.0  # x pre-scale
WS = 64.0  # weight pre-scale
NKP = HIDDEN // 256  # 8 contraction chunk-pairs
NSC = S // 512  # 4 sequence chunks
NST = S // 128  # 16 sequence tiles
QKV_O = 3 * HPC * DH  # 1536

_CACHE = {}

CFG = {
    "x_bufs": 2, "rope_bufs": 4, "ps_bufs": 8,
    "pm_bufs": 6, "pr_bufs": 6, "strip_bufs": 2,
    "pss_bufs": 2, "pst_bufs": 2, "pso_bufs": 2,
    "psc_bufs": 4, "ob_bufs": 6,
    "phases": "ABC",
}


def _build_module(repeat=1, cfg=None):
    cfg = {**CFG, **(cfg or {})}
    import concourse.tile as tile
    from concourse import bacc, mybir
    from contextlib import ExitStack

    f32 = mybir.dt.float32
    bf16 = mybir.dt.bfloat16
    f8 = mybir.dt.float8e4
    AF = mybir.ActivationFunctionType
    DR = mybir.MatmulPerfMode.DoubleRow
    ALU = mybir.AluOpType

    nc = bacc.Bacc("TRN2", target_bir_lowering=False, debug=False)

    # packed fp8 inputs (see make_in_maps for layouts)
    x8_d = nc.declare_dram_parameter("x8", [NSC, NKP, 128, 2, 2, 512], f8, isOutput=False)
    w8_d = nc.declare_dram_parameter("w8", [NKP, 128, 2, 2, QKV_O], f8, isOutput=False)
    wo8_d = nc.declare_dram_parameter("wo8", [128, HPC, 2, HIDDEN], f8, isOutput=False)
    cosq = nc.declare_dram_parameter("cosq", [DH, S], bf16, isOutput=False)
    sinq = nc.declare_dram_parameter("sinq", [DH, S], bf16, isOutput=False)
    cosk = nc.declare_dram_parameter("cosk", [DH, S], bf16, isOutput=False)
    sink = nc.declare_dram_parameter("sink", [DH, S], bf16, isOutput=False)
    maskb_d = nc.declare_dram_parameter("maskb", [128, 256], bf16, isOutput=False)
    idnb_d = nc.declare_dram_parameter("idnb", [128, 128], bf16, isOutput=False)
    out_d = nc.declare_dram_parameter("out", [S, HIDDEN], bf16, isOutput=True)

    with tile.TileContext(nc) as tc, ExitStack() as top:
        cpool = top.enter_context(tc.tile_pool(name="consts", bufs=1))
        msk = cpool.tile([128, 256], bf16, tag="mask")
        nc.sync.dma_start(msk[:], maskb_d[:])
        idnb = cpool.tile([128, 128], bf16, tag="idnb")
        nc.sync.dma_start(idnb[:], idnb_d[:])
        tb = {}
        tbl_srcs = (("cosq", cosq), ("sinq", sinq), ("cosk", cosk), ("sink", sink))
        for nm, _s in tbl_srcs:
            t = cpool.tile([128, S], bf16, tag=nm)
            tb[nm] = t

        # rope'd q/k and v stay in SBUF across phases
        qkv_pool = top.enter_context(tc.tile_pool(name="qkv", bufs=1))
        attn_pool = top.enter_context(tc.tile_pool(name="attn", bufs=1))

        for rep in range(repeat):
            qk_all = qkv_pool.tile([128, 2 * HPC, S], bf16, tag="qk_all")
            v_all = qkv_pool.tile([128, NST, HPC * DH], bf16, tag="v_all")

            # ------------- Phase A: QKV projection + RoPE -------------
            if "A" in cfg["phases"]:
              with ExitStack() as ph:
                w_pool = ph.enter_context(tc.tile_pool(name="wt", bufs=1))
                x_pool = ph.enter_context(tc.tile_pool(name="xt", bufs=cfg["x_bufs"]))
                rope_pool = ph.enter_context(tc.tile_pool(name="rope", bufs=cfg["rope_bufs"]))
                ps_pool = ph.enter_context(
                    tc.tile_pool(name="psa", bufs=cfg["ps_bufs"], space="PSUM")
                )

                # weights: [128, kp, j(hi/lo), c(pair), cols]. DMA order:
                # hi-w/hi-x chunks first (the I1 pass can start after ~0.5MB),
                # then the lo chunks, then the rope tables.
                wt = w_pool.tile([128, NKP, 2, 2, QKV_O], f8, tag="wt")
                xts = {}
                xts[0] = x_pool.tile([128, NKP, 2, 2, 512], f8, tag="xt", name="xt0")
                for kp in range(NKP):
                    nc.sync.dma_start(wt[:, kp, 0], w8_d[kp, :, 0])
                    nc.sync.dma_start(xts[0][:, kp, 0], x8_d[0, kp, :, 0])
                for kp in range(NKP):
                    nc.sync.dma_start(xts[0][:, kp, 1], x8_d[0, kp, :, 1])
                for kp in range(NKP):
                    nc.sync.dma_start(wt[:, kp, 1], w8_d[kp, :, 1])
                if rep == 0:
                    # q tables now (needed by the first rope evacs), k tables
                    # after the sc1 x prefetch (k ropes run later)
                    for nm in ("cosq", "sinq"):
                        nc.sync.dma_start(tb[nm][:], dict(tbl_srcs)[nm][:])

                PRODS = ((0, 0), (1, 0), (0, 1))

                for sc in range(NSC):
                    s0 = sc * 512
                    xt = xts.pop(sc)
                    if sc + 1 < NSC:
                        xts[sc + 1] = x_pool.tile([128, NKP, 2, 2, 512], f8, tag="xt", name=f"xt{sc+1}")
                        for kp in range(NKP):
                            nc.sync.dma_start(xts[sc + 1][:, kp], x8_d[sc + 1, kp])
                    if rep == 0 and sc == 0:
                        for nm in ("cosk", "sink"):
                            nc.sync.dma_start(tb[nm][:], dict(tbl_srcs)[nm][:])

                    def rope_evac(t_o, ps):
                        ct = tb["cosq"] if t_o < HPC else tb["cosk"]
                        st = tb["sinq"] if t_o < HPC else tb["sink"]
                        cs = ct[:, s0:s0 + 512]
                        ss = st[:, s0:s0 + 512]
                        tmp = rope_pool.tile([128, 512], f32, tag="tmp")
                        nc.vector.tensor_mul(tmp[0:64, :], ps[64:128, :], ss[0:64, :])
                        nc.vector.tensor_mul(tmp[64:128, :], ps[0:64, :], ss[64:128, :])
                        qc = rope_pool.tile([128, 512], f32, tag="qc")
                        nc.vector.tensor_mul(qc[:], ps[:], cs)
                        nc.gpsimd.tensor_add(
                            qk_all[:, t_o, s0:s0 + 512], qc[:], tmp[:]
                        )

                    # q/k in two half-waves of 4 psum tiles; within each
                    # half-wave 3 kp-outer passes (I1 hi*hi, I2 lo-x, I3 lo-w)
                    # so the PE follows the hi-first DMA stream during fill
                    for hw_i in range(2):
                        ts = [hw_i * 4 + t for t in range(4)]
                        pss = {
                            t: ps_pool.tile([128, 512], f32, tag="psa", name=f"psq{t}")
                            for t in ts
                        }
                        for pi, (jx, jw) in enumerate(PRODS):
                            for kp in range(NKP):
                                for t_o in ts:
                                    nc.tensor.matmul(
                                        pss[t_o][:],
                                        wt[:, kp, jw, :, t_o * 128:(t_o + 1) * 128],
                                        xt[:, kp, jx, :, :],
                                        start=(pi == 0 and kp == 0),
                                        stop=(pi == 2 and kp == NKP - 1),
                                        perf_mode=DR,
                                    )
                        for t_o in ts:
                            rope_evac(t_o, pss[t_o])
                    # v wave (reuses freed psum banks)
                    for st_i in range(4):
                        psv = ps_pool.tile([128, 512], f32, tag="psa", name="psv")
                        for pi, (jx, jw) in enumerate(PRODS):
                            for kp in range(NKP):
                                nc.tensor.matmul(
                                    psv[:],
                                    xt[:, kp, jx, :, st_i * 128:(st_i + 1) * 128],
                                    wt[:, kp, jw, :, 2 * HPC * 128:3 * HPC * 128],
                                    start=(pi == 0 and kp == 0),
                                    stop=(pi == 2 and kp == NKP - 1),
                                    perf_mode=DR,
                                )
                        nc.scalar.mul(
                            v_all[:, sc * 4 + st_i, :], psv[:], 1.0 / (XS * WS)
                        )

            # ------------- Phase C body ----------------------------------
            def emit_phase_c(ph, ah_hi, ah_lo):
                ob_pool = ph.enter_context(tc.tile_pool(name="ob", bufs=cfg["ob_bufs"]))
                psc_pool = ph.enter_context(
                    tc.tile_pool(name="psc", bufs=cfg["psc_bufs"], space="PSUM")
                )
                wo_pool = ph.enter_context(tc.tile_pool(name="wo", bufs=1))
                wo8 = wo_pool.tile([128, HPC, 2, HIDDEN], f8, tag="wo8")
                for h in range(HPC):
                    nc.sync.dma_start(wo8[:, h], wo8_d[:, h])
                for st_i in range(NST):
                    sb = (st_i * 128, st_i * 128 + 128)
                    for mc in range(HIDDEN // 512):
                        ps = psc_pool.tile([128, 512], f32, tag="psc")
                        first = True
                        for hp in range(HPC // 2):
                            hs = (2 * hp, 2 * hp + 2)
                            for lhs_t, jw in (
                                (ah_hi, 0), (ah_lo, 0), (ah_hi, 1),
                            ):
                                nc.tensor.matmul(
                                    ps[:],
                                    lhs_t[:, hs[0]:hs[1], sb[0]:sb[1]],
                                    wo8[:, hs[0]:hs[1], jw, mc * 512:(mc + 1) * 512],
                                    start=first,
                                    stop=(hp == HPC // 2 - 1 and jw == 1),
                                    perf_mode=DR,
                                )
                                first = False
                        ob = ob_pool.tile([128, 512], bf16, tag="ob")
                        if (st_i * 4 + mc) % 2 == 0:
                            nc.scalar.mul(ob[:], ps[:], 1.0 / (XS * WS))
                        else:
                            nc.vector.tensor_scalar_mul(ob[:], ps[:], 1.0 / (XS * WS))
                        nc.sync.dma_start(
                            out_d[st_i * 128:(st_i + 1) * 128,
                                  mc * 512:(mc + 1) * 512],
                            ob[:],
                        )

            # ------------- Phase B: windowed attention -------------
            if "B" in cfg["phases"]:
              with ExitStack() as ph:
                pm_pool = ph.enter_context(tc.tile_pool(name="pm", bufs=cfg["pm_bufs"]))
                pr_pool = ph.enter_context(tc.tile_pool(name="pr", bufs=cfg["pr_bufs"]))
                sm_pool = ph.enter_context(tc.tile_pool(name="sm", bufs=6))
                strip_pool = ph.enter_context(
                    tc.tile_pool(name="strip", bufs=cfg["strip_bufs"])
                )
                phps = ExitStack()
                pss_pool = phps.enter_context(
                    tc.tile_pool(name="pss", bufs=cfg["pss_bufs"], space="PSUM")
                )
                pst_pool = phps.enter_context(
                    tc.tile_pool(name="pst", bufs=cfg["pst_bufs"], space="PSUM")
                )
                pso_pool = phps.enter_context(
                    tc.tile_pool(name="pso", bufs=cfg["pso_bufs"], space="PSUM")
                )

                LAST_BANK = {0: 3, 1: 7, 2: 11, 3: 15}

                def setup_head(h):
                    # piece-granular PV bookkeeping: piece (jb, c, nxt) in
                    # absolute q columns, ready when block min(jb+4, 15) done
                    pieces_by_ready = {}
                    for jb in range(NST):
                        w0, w1 = jb * 128, min(jb * 128 + 640, S)
                        c = w0
                        while c < w1:
                            nxt = min(w1, (c // 512 + 1) * 512)
                            pieces_by_ready.setdefault(
                                min(jb + 4, NST - 1), []
                            ).append((jb, c, nxt))
                            c = nxt
                    roll = strip_pool.tile(
                        [128, 8 * 640], bf16, tag="roll", name=f"roll{h}"
                    )
                    return dict(
                        h=h, qh=qk_all[:, h, :], kh=qk_all[:, HPC + h, :],
                        pv_banks=[None] * 4, touched=[0] * 4,
                        pieces=pieces_by_ready, roll=roll,
                    )

                def strip_ap(st, jb, c0, c1):
                    base = (jb % 8) * 640
                    return st["roll"][:, base + c0:base + c1]

                def emit_pv_pieces(st, i, ah_hi, ah_lo, hcol):
                    h = st["h"]
                    pv_banks, touched = st["pv_banks"], st["touched"]
                    for jb, c, nxt in st["pieces"].get(i, ()):
                        bk = c // 512
                        first_in_bank = pv_banks[bk] is None
                        if first_in_bank:
                            pv_banks[bk] = pso_pool.tile(
                                [128, 512], f32, tag="pvo", name=f"pvo_h{h}_b{bk}"
                            )
                            touched[bk] = c
                        last = LAST_BANK[bk] == jb
                        # split piece at the already-written boundary so each
                        # matmul's region is uniformly pending/not-pending;
                        # start=True only on the bank's very first matmul
                        # (start zeroes the whole 2KB bank)
                        fresh = max(c, touched[bk])
                        subs = []
                        if c < fresh:
                            subs.append((c, fresh))
                        if fresh < nxt:
                            subs.append((fresh, nxt))
                        touched[bk] = max(touched[bk], nxt)
                        for (a, b) in subs:
                            nc.tensor.matmul(
                                pv_banks[bk][:, a - bk * 512:b - bk * 512],
                                v_all[:, jb, hcol * 128:(hcol + 1) * 128],
                                strip_ap(st, jb, a - jb * 128, b - jb * 128),
                                start=first_in_bank,
                                stop=last and b == nxt,
                                skip_group_check=True,
                            )
                            first_in_bank = False
                        if last:
                            ps = pv_banks[bk]
                            cols = (bk * 512, (bk + 1) * 512)
                            nc.vector.tensor_scalar_mul(
                                ah_hi[:, hcol, cols[0]:cols[1]], ps[:], XS
                            )
                            nc.vector.scalar_tensor_tensor(
                                ah_lo[:, hcol, cols[0]:cols[1]],
                                ps[:], XS,
                                ah_hi[:, hcol, cols[0]:cols[1]],
                                ALU.mult, ALU.subtract,
                            )

                def process_block(st, i, ah_hi, ah_lo, hcol):
                    qh, kh = st["qh"], st["kh"]
                    jlo = max(0, i * 128 - WINDOW)
                    w = i * 128 + 128 - jlo
                    nblk = w // 128
                    ps_s = pss_pool.tile([128, 640], f32, tag="pss")
                    # segment cuts: psum bank boundary + mask boundaries.
                    # start=True zeroes the WHOLE 2KB psum bank, so only the
                    # first matmul touching each bank may set it; later writes
                    # to still-pending regions land as if zeroed.
                    cuts = {0, w, w - 128}
                    if i >= 4:
                        cuts.add(128)
                    if w > 512:
                        cuts.add(512)
                    cuts = sorted(cuts)
                    started_banks = set()
                    for a, b in zip(cuts[:-1], cuts[1:]):
                        bk = a // 512
                        masked = None
                        if a >= w - 128:
                            masked = 128 + (a - (w - 128))  # diag pattern
                        elif i >= 4 and a < 128:
                            masked = a  # window-start pattern
                        if masked is not None:
                            nc.tensor.matmul(
                                ps_s[:, a:b],
                                idnb[:],
                                msk[:, masked:masked + (b - a)],
                                start=bk not in started_banks, stop=False,
                                skip_group_check=True,
                            )
                            started_banks.add(bk)
                            acc = True
                        else:
                            acc = bk in started_banks
                            started_banks.add(bk)
                        nc.tensor.matmul(
                            ps_s[:, a:b],
                            qh[:, i * 128:(i + 1) * 128],
                            kh[:, jlo + a:jlo + b],
                            start=not acc,
                            stop=True,
                            skip_group_check=True,
                        )
                    pm = pm_pool.tile([128, 640], bf16, tag="pm")
                    sums = sm_pool.tile([128, 1], f32, tag="sums")
                    nc.scalar.activation(
                        pm[:, :w], ps_s[:, :w], AF.Exp, accum_out=sums[:]
                    )
                    rc = sm_pool.tile([128, 1], f32, tag="rc")
                    nc.vector.reciprocal(rc[:], sums[:])
                    # all-SBUF operands -> legal on Pool (cannot touch PSUM)
                    pr = pr_pool.tile([128, 640], bf16, tag="pr")
                    nc.gpsimd.tensor_scalar_mul(pr[:, :w], pm[:, :w], rc[:])
                    j0 = jlo // 128
                    ps_t = pst_pool.tile([128, 640], bf16, tag="pst")
                    for z in range(nblk):
                        nc.tensor.matmul(
                            ps_t[:, z * 128:(z + 1) * 128],
                            pr[:, z * 128:(z + 1) * 128],
                            idnb[:],
                            is_transpose=True,
                            start=(z == 0), stop=(z == nblk - 1),
                            skip_group_check=True,
                        )
                    roll = st["roll"]
                    # dest col for z: ((j0+z)%8)*640 + (i-j0-z)*128, which is
                    # base + z*512 within a non-wrapping slot segment: axis-
                    # aligned in a (a=col/512, b=(col%512)/128) view
                    roll4 = roll[:].rearrange("p (a b c) -> p a b c", b=4, c=128)
                    ps4 = ps_t[:].rearrange("p (z o c) -> p z o c", o=1, c=128)
                    z = 0
                    while z < nblk:
                        sl0 = (j0 + z) % 8
                        zlen = min(nblk - z, 8 - sl0)
                        base = sl0 * 640 + (i - j0 - z) * 128
                        a0, b0 = base // 512, (base % 512) // 128
                        nc.vector.tensor_copy(
                            roll4[:, a0:a0 + zlen, b0:b0 + 1, :],
                            ps4[:, z:z + zlen, :, :],
                        )
                        z += zlen
                    emit_pv_pieces(st, i, ah_hi, ah_lo, hcol)

                ah_hi = attn_pool.tile([128, HPC, S], f8, tag="ah_hi")
                ah_lo = attn_pool.tile([128, HPC, S], f8, tag="ah_lo")
                for h in range(HPC):
                    st = setup_head(h)
                    for i in range(NST):
                        process_block(st, i, ah_hi, ah_lo, h)

                phps.close()
                if "C" in cfg["phases"]:
                    emit_phase_c(ph, ah_hi, ah_lo)

    nc.compile()
    return nc


def _get_module(repeat=1, cfg=None):
    key = ("nc", repeat, tuple(sorted((cfg or {}).items())))
    if key not in _CACHE:
        _CACHE[key] = _build_module(repeat, cfg)
    return _CACHE[key]


def _hilo(v, dt):
    hi = np.asarray(v, dtype=dt)
    lo = np.asarray(v - hi.astype(np.float32), dtype=dt)
    return hi, lo


def make_in_maps(hidden_states, cos, sin, w_qkv, w_o):
    import ml_dtypes

    E4 = ml_dtypes.float8_e4m3
    BF = ml_dtypes.bfloat16
    hidden_states = np.asarray(hidden_states, dtype=np.float32)
    cos = np.asarray(cos, dtype=np.float32)
    sin = np.asarray(sin, dtype=np.float32)
    w_qkv = np.asarray(w_qkv, dtype=np.float32)
    w_o = np.asarray(w_o, dtype=np.float32)

    cosT = np.ascontiguousarray(cos.T)  # [DH, S]
    sinT = np.ascontiguousarray(sin.T)
    sinS = sinT.copy()
    sinS[: DH // 2] *= -1.0  # fold rotate_half sign
    inv = 1.0 / (XS * WS)
    cq = (cosT * SCALE * inv).astype(BF)
    sq = (sinS * SCALE * inv).astype(BF)
    ck = (cosT * inv).astype(BF)
    sk = (sinS * inv).astype(BF)

    qi = np.arange(128)[:, None]
    cc = np.arange(128)[None, :]
    # window-start pattern: key j = qblock*128 - 512 + c, disallow c <= qr
    mwin = np.where(cc <= qi, NEG, 0.0)
    # diagonal pattern: key j = qblock*128 + c, disallow c > qr
    mdiag = np.where(cc > qi, NEG, 0.0)
    maskb = np.concatenate([mwin, mdiag], axis=1).astype(BF)
    idnb = np.eye(128, dtype=BF)

    # x8: [sc, kp, p, j, c, t] from xT[h= kp*256 + c*128 + p, s= sc*512 + t]
    x8s = []
    for b in range(B):
        xT = np.ascontiguousarray(hidden_states[b].T) * XS
        xh, xl = _hilo(xT, E4)
        arr = np.stack([xh, xl], axis=0)  # [j, h, s]
        arr = arr.reshape(2, NKP, 2, 128, NSC, 512)  # j, kp, c, p, sc, t
        x8s.append(np.ascontiguousarray(arr.transpose(4, 1, 3, 0, 2, 5)))

    in_maps = []
    for core in range(N_CORES):
        b, hg = divmod(core, N_CORES // B)
        r0 = hg * HPC * DH
        wq = w_qkv[r0:r0 + HPC * DH]
        wk = w_qkv[N_HEADS * DH + r0:N_HEADS * DH + r0 + HPC * DH]
        wv = w_qkv[2 * N_HEADS * DH + r0:2 * N_HEADS * DH + r0 + HPC * DH]
        wTc = np.concatenate([wq, wk, wv], axis=0).T * WS  # [HIDDEN, 1536]
        wh, wl = _hilo(wTc, E4)
        arr = np.stack([wh, wl], axis=0)  # [j, h, m]
        arr = arr.reshape(2, NKP, 2, 128, QKV_O)  # j, kp, c, p, m
        w8 = np.ascontiguousarray(arr.transpose(1, 3, 0, 2, 4))  # kp,p,j,c,m

        woTc = np.ascontiguousarray(w_o[:, r0:r0 + HPC * DH].T) * WS  # [512, 2048]
        oh, ol = _hilo(woTc, E4)
        arr = np.stack([oh, ol], axis=0).reshape(2, HPC, 128, HIDDEN)
        wo8 = np.ascontiguousarray(arr.transpose(2, 1, 0, 3))  # p, h, j, m

        in_maps.append(
            {
                "x8": x8s[b],
                "w8": w8,
                "wo8": wo8,
                "cosq": cq,
                "sinq": sq,
                "cosk": ck,
                "sink": sk,
                "maskb": maskb,
                "idnb": idnb,
            }
        )
    return in_maps


def gather(results):
    out = np.zeros((B, S, HIDDEN), dtype=np.float32)
    for c in range(N_CORES):
        b = c // (N_CORES // B)
        out[b] += np.asarray(results[c]["out"]).astype(np.float32)
    return out


def kernel(hidden_states, cos, sin, w_qkv, w_o):
    from concourse.bass_utils import run_bass_kernel_spmd

    nc = _get_module()
    in_maps = make_in_maps(hidden_states, cos, sin, w_qkv, w_o)
    res = run_bass_kernel_spmd(nc, in_maps, list(range(N_CORES)))
    return gather(res.results)


# revision 15
# speedup vs baseline: 2.6194x; 2.6194x over previous
"""Sliding-window attention (RoPE + QKV proj + windowed softmax attention + o_proj)
for Trainium2, SPMD over 8 NeuronCores.

Sharding: batch (2) x head-groups (4 groups of 4 heads) -> 8 cores.
Each core computes qkv for its 4 heads, windowed attention, and a partial
o_proj (its heads' columns of w_o); host sums the 4 partials per batch.

Precision scheme:
- QKV proj and o_proj run in fp8 e4m3 with DoubleRow perf mode (2x128
  contraction per instruction at 0.5 cyc/row) using a 3-product hi/lo
  residual decomposition: x*w ~= xh*wh + xl*wh + xh*wl, which costs
  0.75 cyc/row effective and has ~bf16-level error (the dropped xl*wl
  term is O(eps^2)).
- Weights are pre-scaled by 64 and x by 4 (host side) so the fp8 lo
  residuals stay above the e4m3 subnormal flush threshold; the 1/256 is
  folded into the rope tables / evacuation scales.
- The attention phase (scores, softmax, PV) runs in bf16.

Rope'd q/k and v stay resident in SBUF between the QKV and attention
phases (no DRAM spill).
"""
import sys

sys.path.insert(0, "/opt/trn_rl_repo")

import numpy as np

B = 2
S = 2048
HIDDEN = 2048
N_HEADS = 16
DH = 128
WINDOW = 512
HPC = 4  # heads per core
N_CORES = 8
SCALE = 1.0 / np.sqrt(DH)
NEG = -1.0e30
XS = 4.0  # x pre-scale
WS = 64.0  # weight pre-scale
NKP = HIDDEN // 256  # 8 contraction chunk-pairs
NSC = S // 512  # 4 sequence chunks
NST = S // 128  # 16 sequence tiles
QKV_O = 3 * HPC * DH  # 1536

_CACHE = {}

CFG = {
    "x_bufs": 2, "rope_bufs": 4, "ps_bufs": 8,
    "pm_bufs": 6, "pr_bufs": 6, "strip_bufs": 2,
    "pss_bufs": 2, "pst_bufs": 2, "pso_bufs": 2,
    "psc_bufs": 4, "ob_bufs": 6,
    "phases": "ABC",
}


def _build_module(repeat=1, cfg=None):
    cfg = {**CFG, **(cfg or {})}
    import concourse.tile as tile
    from concourse import bacc, mybir
    from contextlib import ExitStack

    f32 = mybir.dt.float32
    bf16 = mybir.dt.bfloat16
    f8 = mybir.dt.float8e4
    AF = mybir.ActivationFunctionType
    DR = mybir.MatmulPerfMode.DoubleRow
    ALU = mybir.AluOpType

    nc = bacc.Bacc("TRN2", target_bir_lowering=False, debug=False)

    # packed fp8 inputs (see make_in_maps for layouts)
    x8_d = nc.declare_dram_parameter("x8", [NSC, NKP, 128, 2, 2, 512], f8, isOutput=False)
    w8_d = nc.declare_dram_parameter("w8", [NKP, 128, 2, 2, QKV_O], f8, isOutput=False)
    wo8_d = nc.declare_dram_parameter("wo8", [128, HPC, 2, HIDDEN], f8, isOutput=False)
    cosq = nc.declare_dram_parameter("cosq", [DH, S], bf16, isOutput=False)
    sinq = nc.declare_dram_parameter("sinq", [DH, S], bf16, isOutput=False)
    cosk = nc.declare_dram_parameter("cosk", [DH, S], bf16, isOutput=False)
    sink = nc.declare_dram_parameter("sink", [DH, S], bf16, isOutput=False)
    maskb_d = nc.declare_dram_parameter("maskb", [128, 256], bf16, isOutput=False)
    idnb_d = nc.declare_dram_parameter("idnb", [128, 128], bf16, isOutput=False)
    out_d = nc.declare_dram_parameter("out", [S, HIDDEN], bf16, isOutput=True)

    with tile.TileContext(nc) as tc, ExitStack() as top:
        cpool = top.enter_context(tc.tile_pool(name="consts", bufs=1))
        msk = cpool.tile([128, 256], bf16, tag="mask")
        nc.sync.dma_start(msk[:], maskb_d[:])
        idnb = cpool.tile([128, 128], bf16, tag="idnb")
        nc.sync.dma_start(idnb[:], idnb_d[:])
        tb = {}
        tbl_srcs = (("cosq", cosq), ("sinq", sinq), ("cosk", cosk), ("sink", sink))
        for nm, _s in tbl_srcs:
            t = cpool.tile([128, S], bf16, tag=nm)
            tb[nm] = t

        # rope'd q/k and v stay in SBUF across phases
        qkv_pool = top.enter_context(tc.tile_pool(name="qkv", bufs=1))
        attn_pool = top.enter_context(tc.tile_pool(name="attn", bufs=1))

        for rep in range(repeat):
            qk_all = qkv_pool.tile([128, 2 * HPC, S], bf16, tag="qk_all")
            v_all = qkv_pool.tile([128, NST, HPC * DH], bf16, tag="v_all")

            # ------------- Phase A: QKV projection + RoPE -------------
            if "A" in cfg["phases"]:
              with ExitStack() as ph:
                w_pool = ph.enter_context(tc.tile_pool(name="wt", bufs=1))
                x_pool = ph.enter_context(tc.tile_pool(name="xt", bufs=cfg["x_bufs"]))
                rope_pool = ph.enter_context(tc.tile_pool(name="rope", bufs=cfg["rope_bufs"]))
                ps_pool = ph.enter_context(
                    tc.tile_pool(name="psa", bufs=cfg["ps_bufs"], space="PSUM")
                )

                # weights: [128, kp, j(hi/lo), c(pair), cols]. DMA order:
                # hi-w/hi-x chunks first (the I1 pass can start after ~0.5MB),
                # then the lo chunks, then the rope tables.
                wt = w_pool.tile([128, NKP, 2, 2, QKV_O], f8, tag="wt")
                xts = {}
                xts[0] = x_pool.tile([128, NKP, 2, 2, 512], f8, tag="xt", name="xt0")
                for kp in range(NKP):
                    nc.sync.dma_start(wt[:, kp, 0], w8_d[kp, :, 0])
                    nc.sync.dma_start(xts[0][:, kp, 0], x8_d[0, kp, :, 0])
                for kp in range(NKP):
                    nc.sync.dma_start(xts[0][:, kp, 1], x8_d[0, kp, :, 1])
                for kp in range(NKP):
                    nc.sync.dma_start(wt[:, kp, 1], w8_d[kp, :, 1])
                if rep == 0:
                    # q tables now (needed by the first rope evacs), k tables
                    # after the sc1 x prefetch (k ropes run later)
                    for nm in ("cosq", "sinq"):
                        nc.sync.dma_start(tb[nm][:], dict(tbl_srcs)[nm][:])

                PRODS = ((0, 0), (1, 0), (0, 1))

                for sc in range(NSC):
                    s0 = sc * 512
                    xt = xts.pop(sc)
                    if sc + 1 < NSC:
                        xts[sc + 1] = x_pool.tile([128, NKP, 2, 2, 512], f8, tag="xt", name=f"xt{sc+1}")
                        for kp in range(NKP):
                            nc.sync.dma_start(xts[sc + 1][:, kp], x8_d[sc + 1, kp])
                    if rep == 0 and sc == 0:
                        for nm in ("cosk", "sink"):
                            nc.sync.dma_start(tb[nm][:], dict(tbl_srcs)[nm][:])

                    def rope_evac(t_o, ps):
                        ct = tb["cosq"] if t_o < HPC else tb["cosk"]
                        st = tb["sinq"] if t_o < HPC else tb["sink"]
                        cs = ct[:, s0:s0 + 512]
                        ss = st[:, s0:s0 + 512]
                        tmp = rope_pool.tile([128, 512], f32, tag="tmp")
                        nc.vector.tensor_mul(tmp[0:64, :], ps[64:128, :], ss[0:64, :])
                        nc.vector.tensor_mul(tmp[64:128, :], ps[0:64, :], ss[64:128, :])
                        qc = rope_pool.tile([128, 512], f32, tag="qc")
                        nc.vector.tensor_mul(qc[:], ps[:], cs)
                        nc.gpsimd.tensor_add(
                            qk_all[:, t_o, s0:s0 + 512], qc[:], tmp[:]
                        )

                    # q/k in two half-waves of 4 psum tiles; within each
                    # half-wave 3 kp-outer passes (I1 hi*hi, I2 lo-x, I3 lo-w)
                    # so the PE follows the hi-first DMA stream during fill
                    for hw_i in range(2):
                        ts = [hw_i * 4 + t for t in range(4)]
                        pss = {
                            t: ps_pool.tile([128, 512], f32, tag="psa", name=f"psq{t}")
                            for t in ts
                        }
                        for pi, (jx, jw) in enumerate(PRODS):
                            for kp in range(NKP):
                                for t_o in ts:
                                    nc.tensor.matmul(
                                        pss[t_o][:],
                                        wt[:, kp, jw, :, t_o * 128:(t_o + 1) * 128],
                                        xt[:, kp, jx, :, :],
                                        start=(pi == 0 and kp == 0),
                                        stop=(pi == 2 and kp == NKP - 1),
                                        perf_mode=DR,
                                    )
                        for t_o in ts:
                            rope_evac(t_o, pss[t_o])
                    # v wave (reuses freed psum banks)
                    for st_i in range(4):
                        psv = ps_pool.tile([128, 512], f32, tag="psa", name="psv")
                        for pi, (jx, jw) in enumerate(PRODS):
                            for kp in range(NKP):
                                nc.tensor.matmul(
                                    psv[:],
                                    xt[:, kp, jx, :, st_i * 128:(st_i + 1) * 128],
                                    wt[:, kp, jw, :, 2 * HPC * 128:3 * HPC * 128],
                                    start=(pi == 0 and kp == 0),
                                    stop=(pi == 2 and kp == NKP - 1),
                                    perf_mode=DR,
                                )
                        nc.scalar.mul(
                            v_all[:, sc * 4 + st_i, :], psv[:], 1.0 / (XS * WS)
                        )

            # ------------- Phase C body ----------------------------------
            def emit_phase_c(ph, ah_hi, ah_lo):
                ob_pool = ph.enter_context(tc.tile_pool(name="ob", bufs=cfg["ob_bufs"]))
                psc_pool = ph.enter_context(
                    tc.tile_pool(name="psc", bufs=cfg["psc_bufs"], space="PSUM")
                )
                wo_pool = ph.enter_context(tc.tile_pool(name="wo", bufs=1))
                wo8 = wo_pool.tile([128, HPC, 2, HIDDEN], f8, tag="wo8")
                for h in range(HPC):
                    nc.sync.dma_start(wo8[:, h], wo8_d[:, h])
                for st_i in range(NST):
                    sb = (st_i * 128, st_i * 128 + 128)
                    for mc in range(HIDDEN // 512):
                        ps = psc_pool.tile([128, 512], f32, tag="psc")
                        first = True
                        for hp in range(HPC // 2):
                            hs = (2 * hp, 2 * hp + 2)
                            for lhs_t, jw in (
                                (ah_hi, 0), (ah_lo, 0), (ah_hi, 1),
                            ):
                                nc.tensor.matmul(
                                    ps[:],
                                    lhs_t[:, hs[0]:hs[1], sb[0]:sb[1]],
                                    wo8[:, hs[0]:hs[1], jw, mc * 512:(mc + 1) * 512],
                                    start=first,
                                    stop=(hp == HPC // 2 - 1 and jw == 1),
                                    perf_mode=DR,
                                )
                                first = False
                        ob = ob_pool.tile([128, 512], bf16, tag="ob")
                        if (st_i * 4 + mc) % 2 == 0:
                            nc.scalar.mul(ob[:], ps[:], 1.0 / (XS * WS))
                        else:
                            nc.vector.tensor_scalar_mul(ob[:], ps[:], 1.0 / (XS * WS))
                        nc.sync.dma_start(
                            out_d[st_i * 128:(st_i + 1) * 128,
                                  mc * 512:(mc + 1) * 512],
                            ob[:],
                        )

            # ------------- Phase B: windowed attention -------------
            if "B" in cfg["phases"]:
              with ExitStack() as ph:
                pm_pool = ph.enter_context(tc.tile_pool(name="pm", bufs=cfg["pm_bufs"]))
                pr_pool = ph.enter_context(tc.tile_pool(name="pr", bufs=cfg["pr_bufs"]))
                sm_pool = ph.enter_context(tc.tile_pool(name="sm", bufs=6))
                strip_pool = ph.enter_context(
                    tc.tile_pool(name="strip", bufs=cfg["strip_bufs"])
                )
                phps = ExitStack()
                pss_pool = phps.enter_context(
                    tc.tile_pool(name="pss", bufs=cfg["pss_bufs"], space="PSUM")
                )
                pst_pool = phps.enter_context(
                    tc.tile_pool(name="pst", bufs=cfg["pst_bufs"], space="PSUM")
                )
                pso_pool = phps.enter_context(
                    tc.tile_pool(name="pso", bufs=cfg["pso_bufs"], space="PSUM")
                )

                LAST_BANK = {0: 3, 1: 7, 2: 11, 3: 15}

                def setup_head(h):
                    # piece-granular PV bookkeeping: piece (jb, c, nxt) in
                    # absolute q columns, ready when block min(jb+4, 15) done
                    pieces_by_ready = {}
                    for jb in range(NST):
                        w0, w1 = jb * 128, min(jb * 128 + 640, S)
                        c = w0
                        while c < w1:
                            nxt = min(w1, (c // 512 + 1) * 512)
                            pieces_by_ready.setdefault(
                                min(jb + 4, NST - 1), []
                            ).append((jb, c, nxt))
                            c = nxt
                    roll = strip_pool.tile(
                        [128, 8 * 640], bf16, tag="roll", name=f"roll{h}"
                    )
                    return dict(
                        h=h, qh=qk_all[:, h, :], kh=qk_all[:, HPC + h, :],
                        pv_banks=[None] * 4, touched=[0] * 4,
                        pieces=pieces_by_ready, roll=roll,
                    )

                def strip_ap(st, jb, c0, c1):
                    base = (jb % 8) * 640
                    return st["roll"][:, base + c0:base + c1]

                def emit_pv_pieces(st, i, ah_hi, ah_lo, hcol):
                    h = st["h"]
                    pv_banks, touched = st["pv_banks"], st["touched"]
                    for jb, c, nxt in st["pieces"].get(i, ()):
                        bk = c // 512
                        first_in_bank = pv_banks[bk] is None
                        if first_in_bank:
                            pv_banks[bk] = pso_pool.tile(
                                [128, 512], f32, tag="pvo", name=f"pvo_h{h}_b{bk}"
                            )
                            touched[bk] = c
                        last = LAST_BANK[bk] == jb
                        # split piece at the already-written boundary so each
                        # matmul's region is uniformly pending/not-pending;
                        # start=True only on the bank's very first matmul
                        # (start zeroes the whole 2KB bank)
                        fresh = max(c, touched[bk])
                        subs = []
                        if c < fresh:
                            subs.append((c, fresh))
                        if fresh < nxt:
                            subs.append((fresh, nxt))
                        touched[bk] = max(touched[bk], nxt)
                        for (a, b) in subs:
                            nc.tensor.matmul(
                                pv_banks[bk][:, a - bk * 512:b - bk * 512],
                                v_all[:, jb, hcol * 128:(hcol + 1) * 128],
                                strip_ap(st, jb, a - jb * 128, b - jb * 128),
                                start=first_in_bank,
                                stop=last and b == nxt,
                                skip_group_check=True,
                            )
                            first_in_bank = False
                        if last:
                            ps = pv_banks[bk]
                            cols = (bk * 512, (bk + 1) * 512)
                            nc.scalar.mul(
                                ah_hi[:, hcol, cols[0]:cols[1]], ps[:], XS
                            )
                            nc.vector.scalar_tensor_tensor(
                                ah_lo[:, hcol, cols[0]:cols[1]],
                                ps[:], XS,
                                ah_hi[:, hcol, cols[0]:cols[1]],
                                ALU.mult, ALU.subtract,
                            )

                def process_block(st, i, ah_hi, ah_lo, hcol):
                    qh, kh = st["qh"], st["kh"]
                    jlo = max(0, i * 128 - WINDOW)
                    w = i * 128 + 128 - jlo
                    nblk = w // 128
                    ps_s = pss_pool.tile([128, 640], f32, tag="pss")
                    # segment cuts: psum bank boundary + mask boundaries.
                    # start=True zeroes the WHOLE 2KB psum bank, so only the
                    # first matmul touching each bank may set it; later writes
                    # to still-pending regions land as if zeroed.
                    cuts = {0, w, w - 128}
                    if i >= 4:
                        cuts.add(128)
                    if w > 512:
                        cuts.add(512)
                    cuts = sorted(cuts)
                    started_banks = set()
                    for a, b in zip(cuts[:-1], cuts[1:]):
                        bk = a // 512
                        masked = None
                        if a >= w - 128:
                            masked = 128 + (a - (w - 128))  # diag pattern
                        elif i >= 4 and a < 128:
                            masked = a  # window-start pattern
                        if masked is not None:
                            nc.tensor.matmul(
                                ps_s[:, a:b],
                                idnb[:],
                                msk[:, masked:masked + (b - a)],
                                start=bk not in started_banks, stop=False,
                                skip_group_check=True,
                            )
                            started_banks.add(bk)
                            acc = True
                        else:
                            acc = bk in started_banks
                            started_banks.add(bk)
                        nc.tensor.matmul(
                            ps_s[:, a:b],
                            qh[:, i * 128:(i + 1) * 128],
                            kh[:, jlo + a:jlo + b],
                            start=not acc,
                            stop=True,
                            skip_group_check=True,
                        )
                    pm = pm_pool.tile([128, 640], bf16, tag="pm")
                    sums = sm_pool.tile([128, 1], f32, tag="sums")
                    nc.scalar.activation(
                        pm[:, :w], ps_s[:, :w], AF.Exp, accum_out=sums[:]
                    )
                    rc = sm_pool.tile([128, 1], f32, tag="rc")
                    nc.vector.reciprocal(rc[:], sums[:])
                    # all-bf16 SBUF operands -> DVE 4x mode; keep gpsimd off
                    # the critical chain (Q7 software ops are slow on HW)
                    pr = pr_pool.tile([128, 640], bf16, tag="pr")
                    nc.vector.tensor_scalar_mul(pr[:, :w], pm[:, :w], rc[:])
                    j0 = jlo // 128
                    ps_t = pst_pool.tile([128, 640], bf16, tag="pst")
                    for z in range(nblk):
                        nc.tensor.matmul(
                            ps_t[:, z * 128:(z + 1) * 128],
                            pr[:, z * 128:(z + 1) * 128],
                            idnb[:],
                            is_transpose=True,
                            start=(z == 0), stop=(z == nblk - 1),
                            skip_group_check=True,
                        )
                    roll = st["roll"]
                    # dest col for z: ((j0+z)%8)*640 + (i-j0-z)*128, which is
                    # base + z*512 within a non-wrapping slot segment: axis-
                    # aligned in a (a=col/512, b=(col%512)/128) view
                    roll4 = roll[:].rearrange("p (a b c) -> p a b c", b=4, c=128)
                    ps4 = ps_t[:].rearrange("p (z o c) -> p z o c", o=1, c=128)
                    z = 0
                    while z < nblk:
                        sl0 = (j0 + z) % 8
                        zlen = min(nblk - z, 8 - sl0)
                        base = sl0 * 640 + (i - j0 - z) * 128
                        a0, b0 = base // 512, (base % 512) // 128
                        nc.vector.tensor_copy(
                            roll4[:, a0:a0 + zlen, b0:b0 + 1, :],
                            ps4[:, z:z + zlen, :, :],
                        )
                        z += zlen
                    emit_pv_pieces(st, i, ah_hi, ah_lo, hcol)

                ah_hi = attn_pool.tile([128, HPC, S], f8, tag="ah_hi")
                ah_lo = attn_pool.tile([128, HPC, S], f8, tag="ah_lo")
                for h in range(HPC):
                    st = setup_head(h)
                    for i in range(NST):
                        process_block(st, i, ah_hi, ah_lo, h)

                phps.close()
                if "C" in cfg["phases"]:
                    emit_phase_c(ph, ah_hi, ah_lo)

    nc.compile()
    return nc


def _get_module(repeat=1, cfg=None):
    key = ("nc", repeat, tuple(sorted((cfg or {}).items())))
    if key not in _CACHE:
        _CACHE[key] = _build_module(repeat, cfg)
    return _CACHE[key]


def _hilo(v, dt):
    hi = np.asarray(v, dtype=dt)
    lo = np.asarray(v - hi.astype(np.float32), dtype=dt)
    return hi, lo


def make_in_maps(hidden_states, cos, sin, w_qkv, w_o):
    import ml_dtypes

    E4 = ml_dtypes.float8_e4m3
    BF = ml_dtypes.bfloat16
    hidden_states = np.asarray(hidden_states, dtype=np.float32)
    cos = np.asarray(cos, dtype=np.float32)
    sin = np.asarray(sin, dtype=np.float32)
    w_qkv = np.asarray(w_qkv, dtype=np.float32)
    w_o = np.asarray(w_o, dtype=np.float32)

    cosT = np.ascontiguousarray(cos.T)  # [DH, S]
    sinT = np.ascontiguousarray(sin.T)
    sinS = sinT.copy()
    sinS[: DH // 2] *= -1.0  # fold rotate_half sign
    inv = 1.0 / (XS * WS)
    cq = (cosT * SCALE * inv).astype(BF)
    sq = (sinS * SCALE * inv).astype(BF)
    ck = (cosT * inv).astype(BF)
    sk = (sinS * inv).astype(BF)

    qi = np.arange(128)[:, None]
    cc = np.arange(128)[None, :]
    # window-start pattern: key j = qblock*128 - 512 + c, disallow c <= qr
    mwin = np.where(cc <= qi, NEG, 0.0)
    # diagonal pattern: key j = qblock*128 + c, disallow c > qr
    mdiag = np.where(cc > qi, NEG, 0.0)
    maskb = np.concatenate([mwin, mdiag], axis=1).astype(BF)
    idnb = np.eye(128, dtype=BF)

    # x8: [sc, kp, p, j, c, t] from xT[h= kp*256 + c*128 + p, s= sc*512 + t]
    x8s = []
    for b in range(B):
        xT = np.ascontiguousarray(hidden_states[b].T) * XS
        xh, xl = _hilo(xT, E4)
        arr = np.stack([xh, xl], axis=0)  # [j, h, s]
        arr = arr.reshape(2, NKP, 2, 128, NSC, 512)  # j, kp, c, p, sc, t
        x8s.append(np.ascontiguousarray(arr.transpose(4, 1, 3, 0, 2, 5)))

    in_maps = []
    for core in range(N_CORES):
        b, hg = divmod(core, N_CORES // B)
        r0 = hg * HPC * DH
        wq = w_qkv[r0:r0 + HPC * DH]
        wk = w_qkv[N_HEADS * DH + r0:N_HEADS * DH + r0 + HPC * DH]
        wv = w_qkv[2 * N_HEADS * DH + r0:2 * N_HEADS * DH + r0 + HPC * DH]
        wTc = np.concatenate([wq, wk, wv], axis=0).T * WS  # [HIDDEN, 1536]
        wh, wl = _hilo(wTc, E4)
        arr = np.stack([wh, wl], axis=0)  # [j, h, m]
        arr = arr.reshape(2, NKP, 2, 128, QKV_O)  # j, kp, c, p, m
        w8 = np.ascontiguousarray(arr.transpose(1, 3, 0, 2, 4))  # kp,p,j,c,m

        woTc = np.ascontiguousarray(w_o[:, r0:r0 + HPC * DH].T) * WS  # [512, 2048]
        oh, ol = _hilo(woTc, E4)
        arr = np.stack([oh, ol], axis=0).reshape(2, HPC, 128, HIDDEN)
        wo8 = np.ascontiguousarray(arr.transpose(2, 1, 0, 3))  # p, h, j, m

        in_maps.append(
            {
                "x8": x8s[b],
                "w8": w8,
                "wo8": wo8,
                "cosq": cq,
                "sinq": sq,
                "cosk": ck,
                "sink": sk,
                "maskb": maskb,
                "idnb": idnb,
            }
        )
    return in_maps


def gather(results):
    out = np.zeros((B, S, HIDDEN), dtype=np.float32)
    for c in range(N_CORES):
        b = c // (N_CORES // B)
        out[b] += np.asarray(results[c]["out"]).astype(np.float32)
    return out


def kernel(hidden_states, cos, sin, w_qkv, w_o):
    from concourse.bass_utils import run_bass_kernel_spmd

    nc = _get_module()
    in_maps = make_in_maps(hidden_states, cos, sin, w_qkv, w_o)
    res = run_bass_kernel_spmd(nc, in_maps, list(range(N_CORES)))
    return gather(res.results)


# revision 16
# speedup vs baseline: 2.7136x; 1.0360x over previous
"""Sliding-window attention (RoPE + QKV proj + windowed softmax attention + o_proj)
for Trainium2, SPMD over 8 NeuronCores.

Sharding: batch (2) x head-groups (4 groups of 4 heads) -> 8 cores.
Each core computes qkv for its 4 heads, windowed attention, and a partial
o_proj (its heads' columns of w_o); host sums the 4 partials per batch.

Precision scheme:
- QKV proj and o_proj run in fp8 e4m3 with DoubleRow perf mode (2x128
  contraction per instruction at 0.5 cyc/row) using a 3-product hi/lo
  residual decomposition: x*w ~= xh*wh + xl*wh + xh*wl, which costs
  0.75 cyc/row effective and has ~bf16-level error (the dropped xl*wl
  term is O(eps^2)).
- Weights are pre-scaled by 64 and x by 4 (host side) so the fp8 lo
  residuals stay above the e4m3 subnormal flush threshold; the 1/256 is
  folded into the rope tables / evacuation scales.
- The attention phase (scores, softmax, PV) runs in bf16.

Rope'd q/k and v stay resident in SBUF between the QKV and attention
phases (no DRAM spill).
"""
import sys

sys.path.insert(0, "/opt/trn_rl_repo")

import numpy as np

B = 2
S = 2048
HIDDEN = 2048
N_HEADS = 16
DH = 128
WINDOW = 512
HPC = 4  # heads per core
N_CORES = 8
SCALE = 1.0 / np.sqrt(DH)
NEG = -1.0e30
XS = 4.0  # x pre-scale
WS = 64.0  # weight pre-scale
NKP = HIDDEN // 256  # 8 contraction chunk-pairs
NSC = S // 512  # 4 sequence chunks
NST = S // 128  # 16 sequence tiles
QKV_O = 3 * HPC * DH  # 1536

_CACHE = {}

CFG = {
    "x_bufs": 2, "rope_bufs": 4, "ps_bufs": 8,
    "pm_bufs": 6, "pr_bufs": 6, "strip_bufs": 2,
    "pss_bufs": 2, "pst_bufs": 2, "pso_bufs": 2,
    "psc_bufs": 4, "ob_bufs": 6,
    "phases": "ABC",
}


def _build_module(repeat=1, cfg=None):
    cfg = {**CFG, **(cfg or {})}
    import concourse.tile as tile
    from concourse import bacc, mybir
    from contextlib import ExitStack

    f32 = mybir.dt.float32
    bf16 = mybir.dt.bfloat16
    f8 = mybir.dt.float8e4
    AF = mybir.ActivationFunctionType
    DR = mybir.MatmulPerfMode.DoubleRow
    ALU = mybir.AluOpType

    nc = bacc.Bacc("TRN2", target_bir_lowering=False, debug=False)

    # packed fp8 inputs (see make_in_maps for layouts)
    x8_d = nc.declare_dram_parameter("x8", [NSC, NKP, 128, 2, 2, 512], f8, isOutput=False)
    w8_d = nc.declare_dram_parameter("w8", [NKP, 128, 2, 2, QKV_O], f8, isOutput=False)
    wo8_d = nc.declare_dram_parameter("wo8", [128, HPC, 2, HIDDEN], f8, isOutput=False)
    cosq = nc.declare_dram_parameter("cosq", [DH, S], bf16, isOutput=False)
    sinq = nc.declare_dram_parameter("sinq", [DH, S], bf16, isOutput=False)
    cosk = nc.declare_dram_parameter("cosk", [DH, S], bf16, isOutput=False)
    sink = nc.declare_dram_parameter("sink", [DH, S], bf16, isOutput=False)
    maskb_d = nc.declare_dram_parameter("maskb", [128, 256], bf16, isOutput=False)
    idnb_d = nc.declare_dram_parameter("idnb", [128, 128], bf16, isOutput=False)
    out_d = nc.declare_dram_parameter("out", [S, HIDDEN], bf16, isOutput=True)

    with tile.TileContext(nc) as tc, ExitStack() as top:
        cpool = top.enter_context(tc.tile_pool(name="consts", bufs=1))
        msk = cpool.tile([128, 256], bf16, tag="mask")
        nc.sync.dma_start(msk[:], maskb_d[:])
        idnb = cpool.tile([128, 128], bf16, tag="idnb")
        nc.sync.dma_start(idnb[:], idnb_d[:])
        tb = {}
        tbl_srcs = (("cosq", cosq), ("sinq", sinq), ("cosk", cosk), ("sink", sink))
        for nm, _s in tbl_srcs:
            t = cpool.tile([128, S], bf16, tag=nm)
            tb[nm] = t

        # rope'd q/k and v stay in SBUF across phases
        qkv_pool = top.enter_context(tc.tile_pool(name="qkv", bufs=1))
        attn_pool = top.enter_context(tc.tile_pool(name="attn", bufs=1))

        for rep in range(repeat):
            qk_all = qkv_pool.tile([128, 2 * HPC, S], bf16, tag="qk_all")
            v_all = qkv_pool.tile([128, NST, HPC * DH], bf16, tag="v_all")

            # ------------- Phase A: QKV projection + RoPE -------------
            if "A" in cfg["phases"]:
              with ExitStack() as ph:
                w_pool = ph.enter_context(tc.tile_pool(name="wt", bufs=1))
                x_pool = ph.enter_context(tc.tile_pool(name="xt", bufs=cfg["x_bufs"]))
                rope_pool = ph.enter_context(tc.tile_pool(name="rope", bufs=cfg["rope_bufs"]))
                ps_pool = ph.enter_context(
                    tc.tile_pool(name="psa", bufs=cfg["ps_bufs"], space="PSUM")
                )

                # weights: [128, kp, j(hi/lo), c(pair), cols]. DMA order:
                # hi-w/hi-x chunks first (the I1 pass can start after ~0.5MB),
                # then the lo chunks, then the rope tables.
                wt = w_pool.tile([128, NKP, 2, 2, QKV_O], f8, tag="wt")
                xts = {}
                xts[0] = x_pool.tile([128, NKP, 2, 2, 512], f8, tag="xt", name="xt0")
                for kp in range(NKP):
                    nc.sync.dma_start(wt[:, kp, 0], w8_d[kp, :, 0])
                    nc.sync.dma_start(xts[0][:, kp, 0], x8_d[0, kp, :, 0])
                for kp in range(NKP):
                    nc.sync.dma_start(xts[0][:, kp, 1], x8_d[0, kp, :, 1])
                for kp in range(NKP):
                    nc.sync.dma_start(wt[:, kp, 1], w8_d[kp, :, 1])
                if rep == 0:
                    # q tables now (needed by the first rope evacs), k tables
                    # after the sc1 x prefetch (k ropes run later)
                    for nm in ("cosq", "sinq"):
                        nc.sync.dma_start(tb[nm][:], dict(tbl_srcs)[nm][:])

                PRODS = ((0, 0), (1, 0), (0, 1))

                for sc in range(NSC):
                    s0 = sc * 512
                    xt = xts.pop(sc)
                    if sc + 1 < NSC:
                        xts[sc + 1] = x_pool.tile([128, NKP, 2, 2, 512], f8, tag="xt", name=f"xt{sc+1}")
                        for kp in range(NKP):
                            nc.sync.dma_start(xts[sc + 1][:, kp], x8_d[sc + 1, kp])
                    if rep == 0 and sc == 0:
                        for nm in ("cosk", "sink"):
                            nc.sync.dma_start(tb[nm][:], dict(tbl_srcs)[nm][:])

                    def rope_evac(t_o, ps):
                        ct = tb["cosq"] if t_o < HPC else tb["cosk"]
                        st = tb["sinq"] if t_o < HPC else tb["sink"]
                        cs = ct[:, s0:s0 + 512]
                        ss = st[:, s0:s0 + 512]
                        tmp = rope_pool.tile([128, 512], f32, tag="tmp")
                        nc.vector.tensor_mul(tmp[0:64, :], ps[64:128, :], ss[0:64, :])
                        nc.vector.tensor_mul(tmp[64:128, :], ps[0:64, :], ss[64:128, :])
                        qc = rope_pool.tile([128, 512], f32, tag="qc")
                        nc.vector.tensor_mul(qc[:], ps[:], cs)
                        nc.vector.tensor_add(
                            qk_all[:, t_o, s0:s0 + 512], qc[:], tmp[:]
                        )

                    # q/k in two half-waves of 4 psum tiles; within each
                    # half-wave 3 kp-outer passes (I1 hi*hi, I2 lo-x, I3 lo-w)
                    # so the PE follows the hi-first DMA stream during fill
                    for hw_i in range(2):
                        ts = [hw_i * 4 + t for t in range(4)]
                        pss = {
                            t: ps_pool.tile([128, 512], f32, tag="psa", name=f"psq{t}")
                            for t in ts
                        }
                        for pi, (jx, jw) in enumerate(PRODS):
                            for kp in range(NKP):
                                for t_o in ts:
                                    nc.tensor.matmul(
                                        pss[t_o][:],
                                        wt[:, kp, jw, :, t_o * 128:(t_o + 1) * 128],
                                        xt[:, kp, jx, :, :],
                                        start=(pi == 0 and kp == 0),
                                        stop=(pi == 2 and kp == NKP - 1),
                                        perf_mode=DR,
                                    )
                        for t_o in ts:
                            rope_evac(t_o, pss[t_o])
                    # v wave (reuses freed psum banks)
                    for st_i in range(4):
                        psv = ps_pool.tile([128, 512], f32, tag="psa", name="psv")
                        for pi, (jx, jw) in enumerate(PRODS):
                            for kp in range(NKP):
                                nc.tensor.matmul(
                                    psv[:],
                                    xt[:, kp, jx, :, st_i * 128:(st_i + 1) * 128],
                                    wt[:, kp, jw, :, 2 * HPC * 128:3 * HPC * 128],
                                    start=(pi == 0 and kp == 0),
                                    stop=(pi == 2 and kp == NKP - 1),
                                    perf_mode=DR,
                                )
                        nc.scalar.mul(
                            v_all[:, sc * 4 + st_i, :], psv[:], 1.0 / (XS * WS)
                        )

            # ------------- Phase C body ----------------------------------
            def emit_phase_c(ph, ah_hi, ah_lo):
                ob_pool = ph.enter_context(tc.tile_pool(name="ob", bufs=cfg["ob_bufs"]))
                psc_pool = ph.enter_context(
                    tc.tile_pool(name="psc", bufs=cfg["psc_bufs"], space="PSUM")
                )
                wo_pool = ph.enter_context(tc.tile_pool(name="wo", bufs=1))
                wo8 = wo_pool.tile([128, HPC, 2, HIDDEN], f8, tag="wo8")
                for h in range(HPC):
                    nc.sync.dma_start(wo8[:, h], wo8_d[:, h])
                for st_i in range(NST):
                    sb = (st_i * 128, st_i * 128 + 128)
                    for mc in range(HIDDEN // 512):
                        ps = psc_pool.tile([128, 512], f32, tag="psc")
                        first = True
                        for hp in range(HPC // 2):
                            hs = (2 * hp, 2 * hp + 2)
                            for lhs_t, jw in (
                                (ah_hi, 0), (ah_lo, 0), (ah_hi, 1),
                            ):
                                nc.tensor.matmul(
                                    ps[:],
                                    lhs_t[:, hs[0]:hs[1], sb[0]:sb[1]],
                                    wo8[:, hs[0]:hs[1], jw, mc * 512:(mc + 1) * 512],
                                    start=first,
                                    stop=(hp == HPC // 2 - 1 and jw == 1),
                                    perf_mode=DR,
                                )
                                first = False
                        ob = ob_pool.tile([128, 512], bf16, tag="ob")
                        if (st_i * 4 + mc) % 2 == 0:
                            nc.scalar.mul(ob[:], ps[:], 1.0 / (XS * WS))
                        else:
                            nc.vector.tensor_scalar_mul(ob[:], ps[:], 1.0 / (XS * WS))
                        nc.sync.dma_start(
                            out_d[st_i * 128:(st_i + 1) * 128,
                                  mc * 512:(mc + 1) * 512],
                            ob[:],
                        )

            # ------------- Phase B: windowed attention -------------
            if "B" in cfg["phases"]:
              with ExitStack() as ph:
                pm_pool = ph.enter_context(tc.tile_pool(name="pm", bufs=cfg["pm_bufs"]))
                pr_pool = ph.enter_context(tc.tile_pool(name="pr", bufs=cfg["pr_bufs"]))
                sm_pool = ph.enter_context(tc.tile_pool(name="sm", bufs=6))
                strip_pool = ph.enter_context(
                    tc.tile_pool(name="strip", bufs=cfg["strip_bufs"])
                )
                phps = ExitStack()
                pss_pool = phps.enter_context(
                    tc.tile_pool(name="pss", bufs=cfg["pss_bufs"], space="PSUM")
                )
                pst_pool = phps.enter_context(
                    tc.tile_pool(name="pst", bufs=cfg["pst_bufs"], space="PSUM")
                )
                pso_pool = phps.enter_context(
                    tc.tile_pool(name="pso", bufs=cfg["pso_bufs"], space="PSUM")
                )

                LAST_BANK = {0: 3, 1: 7, 2: 11, 3: 15}

                def setup_head(h):
                    # piece-granular PV bookkeeping: piece (jb, c, nxt) in
                    # absolute q columns, ready when block min(jb+4, 15) done
                    pieces_by_ready = {}
                    for jb in range(NST):
                        w0, w1 = jb * 128, min(jb * 128 + 640, S)
                        c = w0
                        while c < w1:
                            nxt = min(w1, (c // 512 + 1) * 512)
                            pieces_by_ready.setdefault(
                                min(jb + 4, NST - 1), []
                            ).append((jb, c, nxt))
                            c = nxt
                    roll = strip_pool.tile(
                        [128, 8 * 640], bf16, tag="roll", name=f"roll{h}"
                    )
                    return dict(
                        h=h, qh=qk_all[:, h, :], kh=qk_all[:, HPC + h, :],
                        pv_banks=[None] * 4, touched=[0] * 4,
                        pieces=pieces_by_ready, roll=roll,
                    )

                def strip_ap(st, jb, c0, c1):
                    base = (jb % 8) * 640
                    return st["roll"][:, base + c0:base + c1]

                def emit_pv_pieces(st, i, ah_hi, ah_lo, hcol):
                    h = st["h"]
                    pv_banks, touched = st["pv_banks"], st["touched"]
                    for jb, c, nxt in st["pieces"].get(i, ()):
                        bk = c // 512
                        first_in_bank = pv_banks[bk] is None
                        if first_in_bank:
                            pv_banks[bk] = pso_pool.tile(
                                [128, 512], f32, tag="pvo", name=f"pvo_h{h}_b{bk}"
                            )
                            touched[bk] = c
                        last = LAST_BANK[bk] == jb
                        # split piece at the already-written boundary so each
                        # matmul's region is uniformly pending/not-pending;
                        # start=True only on the bank's very first matmul
                        # (start zeroes the whole 2KB bank)
                        fresh = max(c, touched[bk])
                        subs = []
                        if c < fresh:
                            subs.append((c, fresh))
                        if fresh < nxt:
                            subs.append((fresh, nxt))
                        touched[bk] = max(touched[bk], nxt)
                        for (a, b) in subs:
                            nc.tensor.matmul(
                                pv_banks[bk][:, a - bk * 512:b - bk * 512],
                                v_all[:, jb, hcol * 128:(hcol + 1) * 128],
                                strip_ap(st, jb, a - jb * 128, b - jb * 128),
                                start=first_in_bank,
                                stop=last and b == nxt,
                                skip_group_check=True,
                            )
                            first_in_bank = False
                        if last:
                            ps = pv_banks[bk]
                            cols = (bk * 512, (bk + 1) * 512)
                            nc.scalar.mul(
                                ah_hi[:, hcol, cols[0]:cols[1]], ps[:], XS
                            )
                            nc.vector.scalar_tensor_tensor(
                                ah_lo[:, hcol, cols[0]:cols[1]],
                                ps[:], XS,
                                ah_hi[:, hcol, cols[0]:cols[1]],
                                ALU.mult, ALU.subtract,
                            )

                def process_block(st, i, ah_hi, ah_lo, hcol):
                    qh, kh = st["qh"], st["kh"]
                    jlo = max(0, i * 128 - WINDOW)
                    w = i * 128 + 128 - jlo
                    nblk = w // 128
                    ps_s = pss_pool.tile([128, 640], f32, tag="pss")
                    # segment cuts: psum bank boundary + mask boundaries.
                    # start=True zeroes the WHOLE 2KB psum bank, so only the
                    # first matmul touching each bank may set it; later writes
                    # to still-pending regions land as if zeroed.
                    cuts = {0, w, w - 128}
                    if i >= 4:
                        cuts.add(128)
                    if w > 512:
                        cuts.add(512)
                    cuts = sorted(cuts)
                    started_banks = set()
                    for a, b in zip(cuts[:-1], cuts[1:]):
                        bk = a // 512
                        masked = None
                        if a >= w - 128:
                            masked = 128 + (a - (w - 128))  # diag pattern
                        elif i >= 4 and a < 128:
                            masked = a  # window-start pattern
                        if masked is not None:
                            nc.tensor.matmul(
                                ps_s[:, a:b],
                                idnb[:],
                                msk[:, masked:masked + (b - a)],
                                start=bk not in started_banks, stop=False,
                                skip_group_check=True,
                            )
                            started_banks.add(bk)
                            acc = True
                        else:
                            acc = bk in started_banks
                            started_banks.add(bk)
                        nc.tensor.matmul(
                            ps_s[:, a:b],
                            qh[:, i * 128:(i + 1) * 128],
                            kh[:, jlo + a:jlo + b],
                            start=not acc,
                            stop=True,
                            skip_group_check=True,
                        )
                    pm = pm_pool.tile([128, 640], bf16, tag="pm")
                    sums = sm_pool.tile([128, 1], f32, tag="sums")
                    nc.scalar.activation(
                        pm[:, :w], ps_s[:, :w], AF.Exp, accum_out=sums[:]
                    )
                    rc = sm_pool.tile([128, 1], f32, tag="rc")
                    nc.vector.reciprocal(rc[:], sums[:])
                    # all-bf16 SBUF operands -> DVE 4x mode; keep gpsimd off
                    # the critical chain (Q7 software ops are slow on HW)
                    pr = pr_pool.tile([128, 640], bf16, tag="pr")
                    nc.vector.tensor_scalar_mul(pr[:, :w], pm[:, :w], rc[:])
                    j0 = jlo // 128
                    ps_t = pst_pool.tile([128, 640], bf16, tag="pst")
                    for z in range(nblk):
                        nc.tensor.matmul(
                            ps_t[:, z * 128:(z + 1) * 128],
                            pr[:, z * 128:(z + 1) * 128],
                            idnb[:],
                            is_transpose=True,
                            start=(z == 0), stop=(z == nblk - 1),
                            skip_group_check=True,
                        )
                    roll = st["roll"]
                    # dest col for z: ((j0+z)%8)*640 + (i-j0-z)*128, which is
                    # base + z*512 within a non-wrapping slot segment: axis-
                    # aligned in a (a=col/512, b=(col%512)/128) view
                    roll4 = roll[:].rearrange("p (a b c) -> p a b c", b=4, c=128)
                    ps4 = ps_t[:].rearrange("p (z o c) -> p z o c", o=1, c=128)
                    z = 0
                    while z < nblk:
                        sl0 = (j0 + z) % 8
                        zlen = min(nblk - z, 8 - sl0)
                        base = sl0 * 640 + (i - j0 - z) * 128
                        a0, b0 = base // 512, (base % 512) // 128
                        nc.vector.tensor_copy(
                            roll4[:, a0:a0 + zlen, b0:b0 + 1, :],
                            ps4[:, z:z + zlen, :, :],
                        )
                        z += zlen
                    emit_pv_pieces(st, i, ah_hi, ah_lo, hcol)

                ah_hi = attn_pool.tile([128, HPC, S], f8, tag="ah_hi")
                ah_lo = attn_pool.tile([128, HPC, S], f8, tag="ah_lo")
                for h in range(HPC):
                    st = setup_head(h)
                    for i in range(NST):
                        process_block(st, i, ah_hi, ah_lo, h)

                phps.close()
                if "C" in cfg["phases"]:
                    emit_phase_c(ph, ah_hi, ah_lo)

    nc.compile()
    return nc


def _get_module(repeat=1, cfg=None):
    key = ("nc", repeat, tuple(sorted((cfg or {}).items())))
    if key not in _CACHE:
        _CACHE[key] = _build_module(repeat, cfg)
    return _CACHE[key]


def _hilo(v, dt):
    hi = np.asarray(v, dtype=dt)
    lo = np.asarray(v - hi.astype(np.float32), dtype=dt)
    return hi, lo


def make_in_maps(hidden_states, cos, sin, w_qkv, w_o):
    import ml_dtypes

    E4 = ml_dtypes.float8_e4m3
    BF = ml_dtypes.bfloat16
    hidden_states = np.asarray(hidden_states, dtype=np.float32)
    cos = np.asarray(cos, dtype=np.float32)
    sin = np.asarray(sin, dtype=np.float32)
    w_qkv = np.asarray(w_qkv, dtype=np.float32)
    w_o = np.asarray(w_o, dtype=np.float32)

    cosT = np.ascontiguousarray(cos.T)  # [DH, S]
    sinT = np.ascontiguousarray(sin.T)
    sinS = sinT.copy()
    sinS[: DH // 2] *= -1.0  # fold rotate_half sign
    inv = 1.0 / (XS * WS)
    cq = (cosT * SCALE * inv).astype(BF)
    sq = (sinS * SCALE * inv).astype(BF)
    ck = (cosT * inv).astype(BF)
    sk = (sinS * inv).astype(BF)

    qi = np.arange(128)[:, None]
    cc = np.arange(128)[None, :]
    # window-start pattern: key j = qblock*128 - 512 + c, disallow c <= qr
    mwin = np.where(cc <= qi, NEG, 0.0)
    # diagonal pattern: key j = qblock*128 + c, disallow c > qr
    mdiag = np.where(cc > qi, NEG, 0.0)
    maskb = np.concatenate([mwin, mdiag], axis=1).astype(BF)
    idnb = np.eye(128, dtype=BF)

    # x8: [sc, kp, p, j, c, t] from xT[h= kp*256 + c*128 + p, s= sc*512 + t]
    x8s = []
    for b in range(B):
        xT = np.ascontiguousarray(hidden_states[b].T) * XS
        xh, xl = _hilo(xT, E4)
        arr = np.stack([xh, xl], axis=0)  # [j, h, s]
        arr = arr.reshape(2, NKP, 2, 128, NSC, 512)  # j, kp, c, p, sc, t
        x8s.append(np.ascontiguousarray(arr.transpose(4, 1, 3, 0, 2, 5)))

    in_maps = []
    for core in range(N_CORES):
        b, hg = divmod(core, N_CORES // B)
        r0 = hg * HPC * DH
        wq = w_qkv[r0:r0 + HPC * DH]
        wk = w_qkv[N_HEADS * DH + r0:N_HEADS * DH + r0 + HPC * DH]
        wv = w_qkv[2 * N_HEADS * DH + r0:2 * N_HEADS * DH + r0 + HPC * DH]
        wTc = np.concatenate([wq, wk, wv], axis=0).T * WS  # [HIDDEN, 1536]
        wh, wl = _hilo(wTc, E4)
        arr = np.stack([wh, wl], axis=0)  # [j, h, m]
        arr = arr.reshape(2, NKP, 2, 128, QKV_O)  # j, kp, c, p, m
        w8 = np.ascontiguousarray(arr.transpose(1, 3, 0, 2, 4))  # kp,p,j,c,m

        woTc = np.ascontiguousarray(w_o[:, r0:r0 + HPC * DH].T) * WS  # [512, 2048]
        oh, ol = _hilo(woTc, E4)
        arr = np.stack([oh, ol], axis=0).reshape(2, HPC, 128, HIDDEN)
        wo8 = np.ascontiguousarray(arr.transpose(2, 1, 0, 3))  # p, h, j, m

        in_maps.append(
            {
                "x8": x8s[b],
                "w8": w8,
                "wo8": wo8,
                "cosq": cq,
                "sinq": sq,
                "cosk": ck,
                "sink": sk,
                "maskb": maskb,
                "idnb": idnb,
            }
        )
    return in_maps


def gather(results):
    out = np.zeros((B, S, HIDDEN), dtype=np.float32)
    for c in range(N_CORES):
        b = c // (N_CORES // B)
        out[b] += np.asarray(results[c]["out"]).astype(np.float32)
    return out


def kernel(hidden_states, cos, sin, w_qkv, w_o):
    from concourse.bass_utils import run_bass_kernel_spmd

    nc = _get_module()
    in_maps = make_in_maps(hidden_states, cos, sin, w_qkv, w_o)
    res = run_bass_kernel_spmd(nc, in_maps, list(range(N_CORES)))
    return gather(res.results)
